# revision 9
# baseline (speedup 1.0000x reference)
"""Trainium2 Bass kernel for relative-position multi-head attention.

Math (derived from the reference, verified numerically):
  The (L,L,depth) relative tensors ak/av are rank-2 in [sin,cos] positional
  features, and the skew unroll is ak[i,j] = a[j-i+L-1].  With
  c = 1.5708/L, sin/cos addition formulas give:

    scores[i,j] = qh_b[i]·kh[j] + A[i]·sin(cj) + B[i]·cos(cj) (+ row-const)
      A = u0*cos_i + u1*sin_i,  B = u1*cos_i - u0*sin_i,  u = qh_b @ Wak^T
    (row-constant terms - including all k-side biases - cancel in softmax)

    out2[i] = P'[i]*Wav[0] + Q'[i]*Wav[1] + bav,   with
      P' = cos_i*Ss - sin_i*Sc,  Q' = cos_i*Sc + sin_i*Ss,
      Ss/Sc = attn-weighted sums of sin_j/cos_j  (extra value columns)

  so the whole relative machinery is +2 contraction rows on the QK matmul
  and +5 value columns (sin,cos,sin,cos,ones) on the PV matmul, plus a tiny
  rank-2 PSUM-accumulated correction.  bv/bav/bd fold into one output bias
  vector; softmax needs no max-subtraction (|scores/64| < ~1).

Sharding: data-parallel, no collectives.  Core ci handles batch ci//2 and
query-half ci%2 (512 queries), all 16 heads, and writes its own output rows.
"""

import numpy as np

B, L, D, H, DEPTH = 4, 1024, 1024, 16, 64
IH = 512            # queries per core
CFREQ = 1.5708 / L  # positional frequency (reference uses literal 1.5708)
NCORES = 8

_PROGRAM_CACHE = {}


def _build_program():
    import concourse.bacc as bacc
    import concourse.bass as bass
    import concourse.mybir as mybir
    import concourse.tile as tile
    from contextlib import ExitStack

    f32 = mybir.dt.float32
    f32r = mybir.dt.float32r
    bf16 = mybir.dt.bfloat16
    AF = mybir.ActivationFunctionType
    Alu = mybir.AluOpType

    def r(ap):  # fp32 tile -> fp32r view for full-rate matmul
        return ap.bitcast(f32r)

    nc = bacc.Bacc("TRN2", target_bir_lowering=False, debug=False)

    # ---- DRAM I/O ----
    q_d = nc.dram_tensor("q", (IH, D), f32r, kind="ExternalInput")
    k_d = nc.dram_tensor("k", (L, D), f32r, kind="ExternalInput")
    x_d = nc.dram_tensor("x", (L, D), f32r, kind="ExternalInput")
    wq_d = nc.dram_tensor("Wq", (D, D), f32r, kind="ExternalInput")
    wk_d = nc.dram_tensor("Wk", (D, D), f32r, kind="ExternalInput")
    wv_d = nc.dram_tensor("Wv", (D, D), f32r, kind="ExternalInput")
    wd_d = nc.dram_tensor("Wd", (D, D), f32r, kind="ExternalInput")
    bq_d = nc.dram_tensor("bq_cols", (128, 8), f32, kind="ExternalInput")
    g4_d = nc.dram_tensor("G4", (64, 4), bf16, kind="ExternalInput")
    wav_d = nc.dram_tensor("Wav_t", (2, 64), bf16, kind="ExternalInput")
    trow_d = nc.dram_tensor("trig_row2", (2, L), bf16, kind="ExternalInput")
    tq4_d = nc.dram_tensor("trigq4", (4, IH), f32, kind="ExternalInput")
    tp4_d = nc.dram_tensor("trigP4", (4, IH), f32, kind="ExternalInput")
    tcols_d = nc.dram_tensor("trig_cols_rep", (128, 8 * 64), bf16, kind="ExternalInput")
    ones42_d = nc.dram_tensor("ones42", (4, 2), f32r, kind="ExternalInput")
    ones64_d = nc.dram_tensor("ones1x64", (1, 64), f32r, kind="ExternalInput")
    cv_d = nc.dram_tensor("cv128", (128, D), f32, kind="ExternalInput")
    id_d = nc.dram_tensor("ident", (128, 128), f32r, kind="ExternalInput")
    out_d = nc.dram_tensor("out", (IH, D), f32, kind="ExternalOutput")

    VSLOT = 100  # per-head value cols: 64 v | ones@64 (pad 65..95) | sin cos sin cos @96..99 (32-aligned PSUM rows)

    with tile.TileContext(nc) as tc, ExitStack() as top:
        # ---- persistent small constants ----
        cpool = top.enter_context(tc.tile_pool(name="consts", bufs=1))
        ident = cpool.tile([128, 128], f32r)
        nc.sync.dma_start(ident[:], id_d.ap())
        g4_t = cpool.tile([64, 4], bf16)
        nc.sync.dma_start(g4_t[:], g4_d.ap())
        wav_t = cpool.tile([2, 64], bf16)
        nc.sync.dma_start(wav_t[:], wav_d.ap())
        trow_t = cpool.tile([2, L], bf16)
        nc.sync.dma_start(trow_t[:], trow_d.ap())
        tq4_t = cpool.tile([4, IH], f32)
        nc.sync.dma_start(tq4_t[:], tq4_d.ap())
        tp4_t = cpool.tile([4, IH], f32)
        nc.sync.dma_start(tp4_t[:], tp4_d.ap())
        tcols_t = cpool.tile([128, 8 * 64], bf16)
        nc.sync.dma_start(tcols_t[:], tcols_d.ap())
        ones42_t = cpool.tile([4, 2], f32r)
        nc.sync.dma_start(ones42_t[:], ones42_d.ap())
        ones64_t = cpool.tile([1, 64], f32r)
        nc.sync.dma_start(ones64_t[:], ones64_d.ap())
        bq_t = cpool.tile([128, 8], f32)
        nc.sync.dma_start(bq_t[:], bq_d.ap())

        # ---- persistent activation/aug tiles ----
        aug_pool = top.enter_context(tc.tile_pool(name="aug", bufs=1))
        k_aug = [aug_pool.tile([66, L], bf16, name=f"k_aug{h}") for h in range(H)]
        q_aug = [aug_pool.tile([66, IH], bf16, name=f"q_aug{h}") for h in range(H)]
        v_int = [aug_pool.tile([128, H * VSLOT], bf16, name=f"v_int{jb}") for jb in range(8)]
        oh_pair = [aug_pool.tile([128, IH], f32r, name=f"oh{p}") for p in range(8)]

        # =========== Phase A+B: transpose inputs & project ===========
        # Transposed copies of q/k/x (contraction dim on partitions).
        def transpose_in(dram, n_l, name, trpool, trpsum, natpool):
            "load (n_l*128, D) natural, return list of 8 tiles (128, n_l*128) = X^T chunks"
            tiles = [trpool.tile([128, n_l * 128], f32r, name=f"{name}T{dc}") for dc in range(8)]
            for lb in range(n_l):
                nat = natpool.tile([128, D], f32r, tag="nat")
                nc.sync.dma_start(nat[:], dram.ap()[lb * 128:(lb + 1) * 128, :])
                for dc in range(8):
                    ps = trpsum.tile([128, 128], f32r, tag="trps")
                    nc.tensor.transpose(ps[:], nat[:, dc * 128:(dc + 1) * 128], ident[:])
                    nc.vector.tensor_copy(tiles[dc][:, lb * 128:(lb + 1) * 128], ps[:])
            return tiles

        with ExitStack() as phb:
            natpool = phb.enter_context(tc.tile_pool(name="nat", bufs=3))
            trpsum = phb.enter_context(tc.tile_pool(name="trps", bufs=2, space="PSUM"))
            wpool = phb.enter_context(tc.tile_pool(name="wts", bufs=8))
            prps = phb.enter_context(tc.tile_pool(name="prps", bufs=2, space="PSUM"))
            smps = phb.enter_context(tc.tile_pool(name="smps", bufs=2, space="PSUM"))
            abps = phb.enter_context(tc.tile_pool(name="abps", bufs=2, space="PSUM"))
            tmp_pool = phb.enter_context(tc.tile_pool(name="abtmp", bufs=2))

            # ---- q: transpose + project + build q_aug ----
            with tc.tile_pool(name="qT", bufs=1) as qTpool:
                qT = transpose_in(q_d, 4, "q", qTpool, trpsum, natpool)
                wq_t = [wpool.tile([128, D], f32r, tag="w", name=f"wq{dc}") for dc in range(8)]
                for dc in range(8):
                    nc.sync.dma_start(wq_t[dc][:], wq_d.ap()[dc * 128:(dc + 1) * 128, :])
                for eb in range(8):
                    ps = prps.tile([128, IH], f32, tag="prj")
                    for dc in range(8):
                        nc.tensor.matmul(
                            ps[:], wq_t[dc][:, eb * 128:(eb + 1) * 128], qT[dc][:],
                            start=(dc == 0), stop=(dc == 7))
                    for s in range(2):  # the two heads in this e-block
                        h = 2 * eb + s
                        nc.vector.tensor_scalar_add(
                            q_aug[h][0:64, :], ps[64 * s:64 * s + 64, :],
                            bq_t[64 * s:64 * s + 64, eb:eb + 1])
                # A/B rows via u4 matmul + elementwise + ones42 matmul
                for h in range(H):
                    u4 = smps.tile([4, IH], f32, tag="u4")
                    nc.tensor.matmul(u4[:], g4_t[:], q_aug[h][0:64, :], start=True, stop=True)
                    t4 = tmp_pool.tile([4, IH], f32r, tag="t4")
                    nc.vector.tensor_mul(t4[:], u4[:], tq4_t[:])
                    ab = abps.tile([2, IH], f32, tag="ab")
                    nc.tensor.matmul(ab[:], ones42_t[:], t4[:], start=True, stop=True)
                    nc.vector.tensor_copy(q_aug[h][64:66, :], ab[:])

            # ---- k: transpose + project + build k_aug ----
            with tc.tile_pool(name="kT", bufs=1) as kTpool:
                kT = transpose_in(k_d, 8, "k", kTpool, trpsum, natpool)
                wk_t = [wpool.tile([128, D], f32r, tag="w", name=f"wk{dc}") for dc in range(8)]
                for dc in range(8):
                    nc.sync.dma_start(wk_t[dc][:], wk_d.ap()[dc * 128:(dc + 1) * 128, :])
                for eb in range(8):
                    for nh in range(2):
                        ps = prps.tile([128, 512], f32, tag="prj")
                        for dc in range(8):
                            nc.tensor.matmul(
                                ps[:],
                                wk_t[dc][:, eb * 128:(eb + 1) * 128],
                                kT[dc][:, nh * 512:(nh + 1) * 512],
                                start=(dc == 0), stop=(dc == 7))
                        for s in range(2):
                            h = 2 * eb + s
                            nc.vector.tensor_copy(
                                k_aug[h][0:64, nh * 512:(nh + 1) * 512],
                                ps[64 * s:64 * s + 64, :])
                for h in range(H):
                    nc.vector.tensor_copy(k_aug[h][64:66, :], trow_t[:])

            # ---- x: transpose + project v + build v_int ----
            with tc.tile_pool(name="xT", bufs=1) as xTpool:
                xT = transpose_in(x_d, 8, "x", xTpool, trpsum, natpool)
                wv_t = [wpool.tile([128, D], f32r, tag="w", name=f"wv{dc}") for dc in range(8)]
                for dc in range(8):
                    nc.sync.dma_start(wv_t[dc][:], wv_d.ap()[dc * 128:(dc + 1) * 128, :])
                for jb in range(8):
                    nc.vector.memset(v_int[jb][:], 1.0)
                    vi3 = v_int[jb][:].rearrange("p (h c) -> p h c", h=H)
                    for nh in range(2):
                        ps = prps.tile([128, 512], f32, tag="prj")
                        for dc in range(8):
                            nc.tensor.matmul(
                                ps[:],
                                xT[dc][:, jb * 128:(jb + 1) * 128],
                                wv_t[dc][:, nh * 512:(nh + 1) * 512],
                                start=(dc == 0), stop=(dc == 7))
                        nc.vector.tensor_copy(
                            vi3[:, 8 * nh:8 * nh + 8, 0:64],
                            ps[:].rearrange("p (h c) -> p h c", h=8))
                    nc.vector.tensor_copy(
                        vi3[:, :, 96:100],
                        tcols_t[:, jb * 64:(jb + 1) * 64].rearrange("p (h c) -> p h c", h=H))

        # =========== Phase C: attention, 2 heads at a time ===========
        with ExitStack() as phc:
            spsum = phc.enter_context(tc.tile_pool(name="spsum", bufs=2, space="PSUM"))
            opsum = phc.enter_context(tc.tile_pool(name="opsum", bufs=2, space="PSUM"))
            pqps = phc.enter_context(tc.tile_pool(name="pqps", bufs=1, space="PSUM"))
            rps = phc.enter_context(tc.tile_pool(name="rps", bufs=1, space="PSUM"))
            epool = phc.enter_context(tc.tile_pool(name="etile", bufs=3))
            cor_pool = phc.enter_context(tc.tile_pool(name="cor", bufs=2))

            for hp in range(8):
                h0, h1 = 2 * hp, 2 * hp + 1
                o_ps = [opsum.tile([VSLOT, IH], f32, tag="ops", name=f"o{h}") for h in (h0, h1)]
                for jb in range(8):
                    s2 = spsum.tile([128, 2 * IH], f32, tag="s2")
                    e2 = epool.tile([128, 2 * IH], bf16, tag="e2")
                    for s, h in enumerate((h0, h1)):
                        nc.tensor.matmul(
                            s2[:, s * IH:(s + 1) * IH],
                            k_aug[h][:, jb * 128:(jb + 1) * 128], q_aug[h][:],
                            start=True, stop=True)
                    nc.scalar.activation(e2[:], s2[:], AF.Exp, scale=1.0 / DEPTH)
                    for s, h in enumerate((h0, h1)):
                        nc.tensor.matmul(
                            o_ps[s][:],
                            v_int[jb][:, h * VSLOT:(h + 1) * VSLOT],
                            e2[:, s * IH:(s + 1) * IH],
                            start=(jb == 0), stop=(jb == 7))
                for s, h in enumerate((h0, h1)):
                    ops = o_ps[s]
                    # P'/Q' correction, accumulated into rows 0:64 on PE
                    t4 = cor_pool.tile([4, IH], f32r, tag="pqt")
                    nc.vector.tensor_mul(t4[:], ops[96:100, :], tp4_t[:])
                    pq = pqps.tile([2, IH], f32, tag="pq")
                    nc.tensor.matmul(pq[:], ones42_t[:], t4[:], start=True, stop=True)
                    pq_sb = cor_pool.tile([2, IH], bf16, tag="pqsb")
                    nc.vector.tensor_copy(pq_sb[:], pq[:])
                    nc.tensor.matmul(ops[0:64, :], wav_t[:], pq_sb[:],
                                     start=False, stop=True, skip_group_check=True)
                    # 1/rowsum, broadcast to 64 partitions via K=1 matmul
                    rinv = cor_pool.tile([1, IH], f32, tag="rinv")
                    nc.vector.reciprocal(rinv[:], ops[64:65, :])
                    rinv_r = cor_pool.tile([1, IH], f32r, tag="rinvr")
                    nc.vector.tensor_copy(rinv_r[:], rinv[:])
                    r64 = rps.tile([64, IH], f32, tag="r64")
                    nc.tensor.matmul(r64[:], ones64_t[:], rinv_r[:], start=True, stop=True)
                    r64_sb = cor_pool.tile([64, IH], f32, tag="r64sb")
                    nc.vector.tensor_copy(r64_sb[:], r64[:])
                    nc.vector.tensor_mul(oh_pair[hp][64 * s:64 * s + 64, :], ops[0:64, :], r64_sb[:])

        # =========== Phase D: output dense + bias + mish ===========
        with ExitStack() as phd:
            wdpool = phd.enter_context(tc.tile_pool(name="wd", bufs=8))
            cvpool = phd.enter_context(tc.tile_pool(name="cv", bufs=1))
            zps = phd.enter_context(tc.tile_pool(name="zps", bufs=2, space="PSUM"))
            mpool = phd.enter_context(tc.tile_pool(name="mish", bufs=2))

            cv_t = cvpool.tile([128, D], f32)
            nc.sync.dma_start(cv_t[:], cv_d.ap())
            wd_t = [wdpool.tile([128, D], f32r, tag="wd", name=f"wd{p}") for p in range(8)]
            for p in range(8):
                nc.sync.dma_start(wd_t[p][:], wd_d.ap()[p * 128:(p + 1) * 128, :])

            for ib in range(4):
                z = zps.tile([128, D], f32, tag="z")
                for p in range(8):
                    for nh in range(2):
                        nc.tensor.matmul(
                            z[:, nh * 512:(nh + 1) * 512],
                            oh_pair[p][:, ib * 128:(ib + 1) * 128],
                            wd_t[p][:, nh * 512:(nh + 1) * 512],
                            start=(p == 0), stop=(p == 7))
                zb = mpool.tile([128, D], f32, tag="zb")
                nc.vector.tensor_add(zb[:], z[:], cv_t[:])
                # mish(z) = z * (1 - 2/(1 + (1+e^z)^2))
                ez = mpool.tile([128, D], f32, tag="ez")
                nc.scalar.activation(ez[:], zb[:], AF.Exp)
                sq = mpool.tile([128, D], f32, tag="sq")
                nc.scalar.activation(sq[:], ez[:], AF.Square, bias=1.0)
                den = mpool.tile([128, D], f32, tag="den")
                nc.vector.tensor_scalar_add(den[:], sq[:], 1.0)
                rec = mpool.tile([128, D], f32, tag="rec")
                nc.vector.reciprocal(rec[:], den[:])
                w = mpool.tile([128, D], f32, tag="wmul")
                nc.vector.tensor_scalar(w[:], rec[:], -2.0, 1.0, Alu.mult, Alu.add)
                res = mpool.tile([128, D], f32, tag="res")
                nc.vector.tensor_mul(res[:], zb[:], w[:])
                nc.sync.dma_start(out_d.ap()[ib * 128:(ib + 1) * 128, :], res[:])

    nc.compile()
    return nc


def _host_inputs(x, k, q, Wq, bq, Wk, bk, Wv, bv, Wak, bak, Wav, bav, Wd, bd):
    """Build the per-core input dicts (pure numpy, constant-folding only)."""
    f32 = np.float32
    bf16 = np.dtype("bfloat16") if hasattr(np, "bfloat16") else None
    import ml_dtypes
    bf16 = ml_dtypes.bfloat16

    def group(W):  # (H, D, DEPTH) -> (D, H*DEPTH)
        return np.ascontiguousarray(W.transpose(1, 0, 2).reshape(D, H * DEPTH)).astype(f32)

    W2q, W2k, W2v = group(Wq), group(Wk), group(Wv)
    bq_cols = np.ascontiguousarray(bq.reshape(H * DEPTH).reshape(8, 128).T).astype(f32)  # (128, 8)

    pos = np.arange(L, dtype=np.float64)
    sin_j = np.sin(CFREQ * pos)
    cos_j = np.cos(CFREQ * pos)
    trig_row2 = np.stack([sin_j, cos_j]).astype(bf16)  # (2, L)

    # per-jb column tiles, replicated per head slot: (128, 8*64)
    tc = np.zeros((128, 8, 16, 4), dtype=np.float64)
    for jb in range(8):
        s = sin_j[jb * 128:(jb + 1) * 128]
        cc = cos_j[jb * 128:(jb + 1) * 128]
        tc[:, jb, :, 0] = s[:, None]
        tc[:, jb, :, 1] = cc[:, None]
        tc[:, jb, :, 2] = s[:, None]
        tc[:, jb, :, 3] = cc[:, None]
    trig_cols_rep = tc.reshape(128, 8 * 64).astype(bf16)

    G4 = np.stack([Wak[0], Wak[1], Wak[0], Wak[1]], axis=1).astype(bf16)  # (64, 4)
    Wav_t = np.asarray(Wav, dtype=bf16)  # (2, 64)
    ones42 = np.array([[1, 0], [1, 0], [0, 1], [0, 1]], dtype=f32)
    ones1x64 = np.ones((1, 64), dtype=f32)
    ident = np.eye(128, dtype=f32)

    bhead = (np.asarray(bv, np.float64) + np.asarray(bav, np.float64)[None, :]).reshape(H * DEPTH)
    cvec = bhead @ np.asarray(Wd, np.float64) + np.asarray(bd, np.float64)
    cv128 = np.broadcast_to(cvec.astype(f32), (128, D)).copy()

    in_maps = []
    for ci in range(NCORES):
        b, ih = ci // 2, ci % 2
        i0 = ih * IH
        ii = pos[i0:i0 + IH]
        sin_i, cos_i = np.sin(CFREQ * ii), np.cos(CFREQ * ii)
        trigq4 = np.stack([cos_i, sin_i, -sin_i, cos_i]).astype(f32)   # for A,B
        trigP4 = np.stack([cos_i, -sin_i, sin_i, cos_i]).astype(f32)   # for P',Q'
        in_maps.append({
            "q": np.ascontiguousarray(q[b, i0:i0 + IH]).astype(f32),
            "k": np.ascontiguousarray(k[b]).astype(f32),
            "x": np.ascontiguousarray(x[b]).astype(f32),
            "Wq": W2q, "Wk": W2k, "Wv": W2v,
            "Wd": np.asarray(Wd, f32),
            "bq_cols": bq_cols,
            "G4": G4, "Wav_t": Wav_t,
            "trig_row2": trig_row2,
            "trigq4": trigq4, "trigP4": trigP4,
            "trig_cols_rep": trig_cols_rep,
            "ones42": ones42, "ones1x64": ones1x64,
            "cv128": cv128, "ident": ident,
        })
    return in_maps


def kernel(**inputs):
    from concourse import bass_utils

    x = np.asarray(inputs["x"]); k = np.asarray(inputs["k"]); q = np.asarray(inputs["q"])
    in_maps = _host_inputs(
        x, k, q,
        np.asarray(inputs["Wq"]), np.asarray(inputs["bq"]),
        np.asarray(inputs["Wk"]), np.asarray(inputs["bk"]),
        np.asarray(inputs["Wv"]), np.asarray(inputs["bv"]),
        np.asarray(inputs["Wak"]), np.asarray(inputs["bak"]),
        np.asarray(inputs["Wav"]), np.asarray(inputs["bav"]),
        np.asarray(inputs["Wd"]), np.asarray(inputs["bd"]),
    )
    if "prog" not in _PROGRAM_CACHE:
        _PROGRAM_CACHE["prog"] = _build_program()
    nc = _PROGRAM_CACHE["prog"]
    res = bass_utils.run_bass_kernel_spmd(nc, in_maps, core_ids=list(range(NCORES)))
    out = np.empty((B, L, D), dtype=np.float32)
    for ci in range(NCORES):
        b, ih = ci // 2, ci % 2
        out[b, ih * IH:(ih + 1) * IH, :] = res.results[ci]["out"]
    return out


# revision 10
# speedup vs baseline: 1.0876x; 1.0876x over previous
"""Trainium2 Bass kernel for relative-position multi-head attention.

Math (derived from the reference, verified numerically):
  The (L,L,depth) relative tensors ak/av are rank-2 in [sin,cos] positional
  features, and the skew unroll is ak[i,j] = a[j-i+L-1].  With
  c = 1.5708/L, sin/cos addition formulas give:

    scores[i,j] = qh_b[i]·kh[j] + A[i]·sin(cj) + B[i]·cos(cj) (+ row-const)
      A = u0*cos_i + u1*sin_i,  B = u1*cos_i - u0*sin_i,  u = qh_b @ Wak^T
    (row-constant terms - including all k-side biases - cancel in softmax)

    out2[i] = P'[i]*Wav[0] + Q'[i]*Wav[1] + bav,   with
      P' = cos_i*Ss - sin_i*Sc,  Q' = cos_i*Sc + sin_i*Ss,
      Ss/Sc = attn-weighted sums of sin_j/cos_j  (extra value columns)

  so the whole relative machinery is +2 contraction rows on the QK matmul
  and +5 value columns on the PV matmul, plus a tiny rank-2 PSUM-accumulated
  correction.  bv/bav/bd fold into one output bias vector; softmax needs no
  max-subtraction (|scores/64| < ~1).  mish(z) = z·(1 - 2/(1+(1+e^z)^2)).

Sharding: data-parallel, no collectives.  Core ci handles batch ci//2 and
query-half ci%2 (512 queries), all 16 heads, and writes its own output rows.
Compute: bf16 matmuls (fp32 PSUM accumulation), fp32 softmax-normalizer path.
Inputs are pre-cast to bf16 on the host; transposed layouts come from
DMA-transpose loads (2-byte dtype requirement satisfied by bf16).
"""

import numpy as np

B, L, D, H, DEPTH = 4, 1024, 1024, 16, 64
IH = 512            # queries per core
CFREQ = 1.5708 / L  # positional frequency (reference uses literal 1.5708)
NCORES = 8

_PROGRAM_CACHE = {}


def _build_program():
    import concourse.bacc as bacc
    import concourse.mybir as mybir
    import concourse.tile as tile
    from contextlib import ExitStack

    f32 = mybir.dt.float32
    f32r = mybir.dt.float32r
    bf16 = mybir.dt.bfloat16
    AF = mybir.ActivationFunctionType
    Alu = mybir.AluOpType

    nc = bacc.Bacc("TRN2", target_bir_lowering=False, debug=False)

    # ---- DRAM I/O ----
    q_d = nc.dram_tensor("q", (IH, D), bf16, kind="ExternalInput")
    k_d = nc.dram_tensor("k", (L, D), bf16, kind="ExternalInput")
    x_d = nc.dram_tensor("x", (L, D), bf16, kind="ExternalInput")
    wq_d = nc.dram_tensor("Wq", (D, D), bf16, kind="ExternalInput")
    wk_d = nc.dram_tensor("Wk", (D, D), bf16, kind="ExternalInput")
    wv_d = nc.dram_tensor("Wv", (D, D), bf16, kind="ExternalInput")
    wd_d = nc.dram_tensor("Wd", (D, D), bf16, kind="ExternalInput")
    bq_d = nc.dram_tensor("bq_cols", (128, 8), f32, kind="ExternalInput")
    g4_d = nc.dram_tensor("G4", (64, 4), bf16, kind="ExternalInput")
    wav_d = nc.dram_tensor("Wav_t", (2, 64), bf16, kind="ExternalInput")
    trow_d = nc.dram_tensor("trig_row2", (2, L), bf16, kind="ExternalInput")
    tq4_d = nc.dram_tensor("trigq4", (4, IH), f32, kind="ExternalInput")
    tp4_d = nc.dram_tensor("trigP4", (4, IH), f32, kind="ExternalInput")
    tcols_d = nc.dram_tensor("trig_cols_rep", (128, 8 * 64), bf16, kind="ExternalInput")
    ones42_d = nc.dram_tensor("ones42", (4, 2), bf16, kind="ExternalInput")
    ones64_d = nc.dram_tensor("ones1x64", (1, 64), f32r, kind="ExternalInput")
    cv_d = nc.dram_tensor("cv128", (128, D), f32, kind="ExternalInput")
    out_d = nc.dram_tensor("out", (IH, D), f32, kind="ExternalOutput")

    VSLOT = 100  # per-head value cols: 64 v | ones@64 (pad) | sin cos sin cos @96..99

    with tile.TileContext(nc) as tc, ExitStack() as top:
        # ---- persistent small constants ----
        cpool = top.enter_context(tc.tile_pool(name="consts", bufs=1))
        g4_t = cpool.tile([64, 4], bf16)
        nc.sync.dma_start(g4_t[:], g4_d.ap())
        wav_t = cpool.tile([2, 64], bf16)
        nc.sync.dma_start(wav_t[:], wav_d.ap())
        trow_t = cpool.tile([2, L], bf16)
        nc.sync.dma_start(trow_t[:], trow_d.ap())
        tq4_t = cpool.tile([4, IH], f32)
        nc.sync.dma_start(tq4_t[:], tq4_d.ap())
        tp4_t = cpool.tile([4, IH], f32)
        nc.sync.dma_start(tp4_t[:], tp4_d.ap())
        tcols_t = cpool.tile([128, 8 * 64], bf16)
        nc.sync.dma_start(tcols_t[:], tcols_d.ap())
        ones42_t = cpool.tile([4, 2], bf16)
        nc.sync.dma_start(ones42_t[:], ones42_d.ap())
        ones64_t = cpool.tile([1, 64], f32r)
        nc.sync.dma_start(ones64_t[:], ones64_d.ap())
        bq_t = cpool.tile([128, 8], f32)
        nc.sync.dma_start(bq_t[:], bq_d.ap())

        # ---- persistent activation/aug tiles ----
        aug_pool = top.enter_context(tc.tile_pool(name="aug", bufs=1))
        k_aug = [aug_pool.tile([66, L], bf16, name=f"k_aug{h}") for h in range(H)]
        q_aug = [aug_pool.tile([66, IH], bf16, name=f"q_aug{h}") for h in range(H)]
        v_int = [aug_pool.tile([128, H * VSLOT], bf16, name=f"v_int{jb}") for jb in range(8)]
        oh_pair = [aug_pool.tile([128, IH], bf16, name=f"oh{p}") for p in range(8)]

        # =========== Phase B: transposed loads + projections ===========
        def transposed_load(dram, n_l, name, trpool):
            "DMA-transpose chunks: tiles[dc] (128, n_l*128) = X[:, dc-block]^T"
            tiles = [trpool.tile([128, n_l * 128], bf16, name=f"{name}T{dc}") for dc in range(8)]
            for dc in range(8):
                nc.sync.dma_start(tiles[dc][:], dram.ap()[:, dc * 128:(dc + 1) * 128],
                                  transpose=True)
            return tiles

        with ExitStack() as phb:
            wpool = phb.enter_context(tc.tile_pool(name="wts", bufs=8))
            prps = phb.enter_context(tc.tile_pool(name="prps", bufs=3, space="PSUM"))
            smps = phb.enter_context(tc.tile_pool(name="smps", bufs=2, space="PSUM"))
            abps = phb.enter_context(tc.tile_pool(name="abps", bufs=2, space="PSUM"))
            tmp_pool = phb.enter_context(tc.tile_pool(name="abtmp", bufs=2))

            # ---- q: load + project + build q_aug ----
            with tc.tile_pool(name="qT", bufs=1) as qTpool:
                qT = transposed_load(q_d, 4, "q", qTpool)
                wq_t = [wpool.tile([128, D], bf16, tag="w", name=f"wq{dc}") for dc in range(8)]
                for dc in range(8):
                    nc.sync.dma_start(wq_t[dc][:], wq_d.ap()[dc * 128:(dc + 1) * 128, :])
                for eb in range(8):
                    ps = prps.tile([128, IH], f32, tag="prj")
                    for dc in range(8):
                        nc.tensor.matmul(
                            ps[:], wq_t[dc][:, eb * 128:(eb + 1) * 128], qT[dc][:],
                            start=(dc == 0), stop=(dc == 7))
                    for s in range(2):  # the two heads in this e-block
                        h = 2 * eb + s
                        nc.vector.tensor_scalar_add(
                            q_aug[h][0:64, :], ps[64 * s:64 * s + 64, :],
                            bq_t[64 * s:64 * s + 64, eb:eb + 1])
                # A/B rows via u4 matmul + elementwise + ones42 matmul
                for h in range(H):
                    u4 = smps.tile([4, IH], f32, tag="u4")
                    nc.tensor.matmul(u4[:], g4_t[:], q_aug[h][0:64, :], start=True, stop=True)
                    t4 = tmp_pool.tile([4, IH], bf16, tag="t4")
                    nc.vector.tensor_mul(t4[:], u4[:], tq4_t[:])
                    ab = abps.tile([2, IH], f32, tag="ab")
                    nc.tensor.matmul(ab[:], ones42_t[:], t4[:], start=True, stop=True)
                    nc.vector.tensor_copy(q_aug[h][64:66, :], ab[:])

            # ---- k: load + project + build k_aug ----
            with tc.tile_pool(name="kT", bufs=1) as kTpool:
                kT = transposed_load(k_d, 8, "k", kTpool)
                wk_t = [wpool.tile([128, D], bf16, tag="w", name=f"wk{dc}") for dc in range(8)]
                for dc in range(8):
                    nc.sync.dma_start(wk_t[dc][:], wk_d.ap()[dc * 128:(dc + 1) * 128, :])
                for eb in range(8):
                    for nh in range(2):
                        ps = prps.tile([128, 512], f32, tag="prj")
                        for dc in range(8):
                            nc.tensor.matmul(
                                ps[:],
                                wk_t[dc][:, eb * 128:(eb + 1) * 128],
                                kT[dc][:, nh * 512:(nh + 1) * 512],
                                start=(dc == 0), stop=(dc == 7))
                        for s in range(2):
                            h = 2 * eb + s
                            nc.vector.tensor_copy(
                                k_aug[h][0:64, nh * 512:(nh + 1) * 512],
                                ps[64 * s:64 * s + 64, :])
                for h in range(H):
                    nc.vector.tensor_copy(k_aug[h][64:66, :], trow_t[:])

            # ---- x: load + project v + build v_int ----
            with tc.tile_pool(name="xT", bufs=1) as xTpool:
                xT = transposed_load(x_d, 8, "x", xTpool)
                wv_t = [wpool.tile([128, D], bf16, tag="w", name=f"wv{dc}") for dc in range(8)]
                for dc in range(8):
                    nc.sync.dma_start(wv_t[dc][:], wv_d.ap()[dc * 128:(dc + 1) * 128, :])
                for jb in range(8):
                    nc.vector.memset(v_int[jb][:], 1.0)
                    vi3 = v_int[jb][:].rearrange("p (h c) -> p h c", h=H)
                    for nh in range(2):
                        ps = prps.tile([128, 512], f32, tag="prj")
                        for dc in range(8):
                            nc.tensor.matmul(
                                ps[:],
                                xT[dc][:, jb * 128:(jb + 1) * 128],
                                wv_t[dc][:, nh * 512:(nh + 1) * 512],
                                start=(dc == 0), stop=(dc == 7))
                        nc.vector.tensor_copy(
                            vi3[:, 8 * nh:8 * nh + 8, 0:64],
                            ps[:].rearrange("p (h c) -> p h c", h=8))
                    nc.vector.tensor_copy(
                        vi3[:, :, 96:100],
                        tcols_t[:, jb * 64:(jb + 1) * 64].rearrange("p (h c) -> p h c", h=H))

        # ====== Phase C: attention, 2 heads at a time, SW-pipelined ======
        with ExitStack() as phc:
            spsum = phc.enter_context(tc.tile_pool(name="spsum", bufs=2, space="PSUM"))
            opsum = phc.enter_context(tc.tile_pool(name="opsum", bufs=2, space="PSUM"))
            pqps = phc.enter_context(tc.tile_pool(name="pqps", bufs=1, space="PSUM"))
            rps = phc.enter_context(tc.tile_pool(name="rps", bufs=1, space="PSUM"))
            epool = phc.enter_context(tc.tile_pool(name="etile", bufs=3))
            cor_pool = phc.enter_context(tc.tile_pool(name="cor", bufs=2))

            for hp in range(8):
                h0, h1 = 2 * hp, 2 * hp + 1
                o_ps = [opsum.tile([VSLOT, IH], f32, tag="ops", name=f"o{h}") for h in (h0, h1)]

                def s_mm(jb):
                    s2 = spsum.tile([128, 2 * IH], f32, tag="s2")
                    for s, h in enumerate((h0, h1)):
                        nc.tensor.matmul(
                            s2[:, s * IH:(s + 1) * IH],
                            k_aug[h][:, jb * 128:(jb + 1) * 128], q_aug[h][:],
                            start=True, stop=True)
                    return s2

                def pv_mm(e2, jb):
                    for s, h in enumerate((h0, h1)):
                        nc.tensor.matmul(
                            o_ps[s][:],
                            v_int[jb][:, h * VSLOT:(h + 1) * VSLOT],
                            e2[:, s * IH:(s + 1) * IH],
                            start=(jb == 0), stop=(jb == 7))

                # software pipeline: S(jb+1) issues before PV(jb) so the PE
                # never waits on the ACT exp of the current block
                pending = None  # (e2, jb)
                s2 = s_mm(0)
                for jb in range(8):
                    e2 = epool.tile([128, 2 * IH], bf16, tag="e2")
                    nc.scalar.activation(e2[:], s2[:], AF.Exp, scale=1.0 / DEPTH)
                    if jb < 7:
                        s2 = s_mm(jb + 1)
                    if pending is not None:
                        pv_mm(*pending)
                    pending = (e2, jb)
                pv_mm(*pending)

                for s, h in enumerate((h0, h1)):
                    ops = o_ps[s]
                    # P'/Q' correction, accumulated into rows 0:64 on PE
                    t4 = cor_pool.tile([4, IH], bf16, tag="pqt")
                    nc.vector.tensor_mul(t4[:], ops[96:100, :], tp4_t[:])
                    pq = pqps.tile([2, IH], f32, tag="pq")
                    nc.tensor.matmul(pq[:], ones42_t[:], t4[:], start=True, stop=True)
                    pq_sb = cor_pool.tile([2, IH], bf16, tag="pqsb")
                    nc.vector.tensor_copy(pq_sb[:], pq[:])
                    nc.tensor.matmul(ops[0:64, :], wav_t[:], pq_sb[:],
                                     start=False, stop=True, skip_group_check=True)
                    # 1/rowsum, broadcast to 64 partitions via K=1 matmul
                    rinv = cor_pool.tile([1, IH], f32, tag="rinv")
                    nc.vector.reciprocal(rinv[:], ops[64:65, :])
                    rinv_r = cor_pool.tile([1, IH], f32r, tag="rinvr")
                    nc.vector.tensor_copy(rinv_r[:], rinv[:])
                    r64 = rps.tile([64, IH], f32, tag="r64")
                    nc.tensor.matmul(r64[:], ones64_t[:], rinv_r[:], start=True, stop=True)
                    r64_sb = cor_pool.tile([64, IH], f32, tag="r64sb")
                    nc.vector.tensor_copy(r64_sb[:], r64[:])
                    nc.vector.tensor_mul(oh_pair[hp][64 * s:64 * s + 64, :], ops[0:64, :], r64_sb[:])

        # =========== Phase D: output dense + bias + mish ===========
        with ExitStack() as phd:
            wdpool = phd.enter_context(tc.tile_pool(name="wd", bufs=8))
            cvpool = phd.enter_context(tc.tile_pool(name="cv", bufs=1))
            zps = phd.enter_context(tc.tile_pool(name="zps", bufs=2, space="PSUM"))
            mpool = phd.enter_context(tc.tile_pool(name="mish", bufs=2))

            cv_t = cvpool.tile([128, D], f32)
            nc.sync.dma_start(cv_t[:], cv_d.ap())
            wd_t = [wdpool.tile([128, D], bf16, tag="wd", name=f"wd{p}") for p in range(8)]
            for p in range(8):
                nc.sync.dma_start(wd_t[p][:], wd_d.ap()[p * 128:(p + 1) * 128, :])

            for ib in range(4):
                z = zps.tile([128, D], f32, tag="z")
                for p in range(8):
                    for nh in range(2):
                        nc.tensor.matmul(
                            z[:, nh * 512:(nh + 1) * 512],
                            oh_pair[p][:, ib * 128:(ib + 1) * 128],
                            wd_t[p][:, nh * 512:(nh + 1) * 512],
                            start=(p == 0), stop=(p == 7))
                zb = mpool.tile([128, D], f32, tag="zb")
                nc.vector.tensor_add(zb[:], z[:], cv_t[:])
                # mish(z) = z * (1 - 2/(1 + (1+e^z)^2))
                ez = mpool.tile([128, D], f32, tag="ez")
                nc.scalar.activation(ez[:], zb[:], AF.Exp)
                sq = mpool.tile([128, D], f32, tag="sq")
                nc.scalar.activation(sq[:], ez[:], AF.Square, bias=1.0)
                den = mpool.tile([128, D], f32, tag="den")
                nc.vector.tensor_scalar_add(den[:], sq[:], 1.0)
                rec = mpool.tile([128, D], f32, tag="rec")
                nc.vector.reciprocal(rec[:], den[:])
                w = mpool.tile([128, D], f32, tag="wmul")
                nc.vector.tensor_scalar(w[:], rec[:], -2.0, 1.0, Alu.mult, Alu.add)
                res = mpool.tile([128, D], f32, tag="res")
                nc.vector.tensor_mul(res[:], zb[:], w[:])
                nc.sync.dma_start(out_d.ap()[ib * 128:(ib + 1) * 128, :], res[:])

    nc.compile()
    return nc


def _host_inputs(x, k, q, Wq, bq, Wk, bk, Wv, bv, Wak, bak, Wav, bav, Wd, bd):
    """Build the per-core input dicts (pure numpy, constant prep only)."""
    import ml_dtypes
    f32 = np.float32
    bf16 = ml_dtypes.bfloat16

    def group(W):  # (H, D, DEPTH) -> (D, H*DEPTH)
        return np.ascontiguousarray(W.transpose(1, 0, 2).reshape(D, H * DEPTH)).astype(bf16)

    W2q, W2k, W2v = group(Wq), group(Wk), group(Wv)
    bq_cols = np.ascontiguousarray(bq.reshape(H * DEPTH).reshape(8, 128).T).astype(f32)

    pos = np.arange(L, dtype=np.float64)
    sin_j = np.sin(CFREQ * pos)
    cos_j = np.cos(CFREQ * pos)
    trig_row2 = np.stack([sin_j, cos_j]).astype(bf16)  # (2, L)

    tcr = np.zeros((128, 8, 16, 4), dtype=np.float64)
    for jb in range(8):
        s = sin_j[jb * 128:(jb + 1) * 128]
        cc = cos_j[jb * 128:(jb + 1) * 128]
        tcr[:, jb, :, 0] = s[:, None]
        tcr[:, jb, :, 1] = cc[:, None]
        tcr[:, jb, :, 2] = s[:, None]
        tcr[:, jb, :, 3] = cc[:, None]
    trig_cols_rep = tcr.reshape(128, 8 * 64).astype(bf16)

    G4 = np.stack([Wak[0], Wak[1], Wak[0], Wak[1]], axis=1).astype(bf16)  # (64, 4)
    Wav_t = np.asarray(Wav, dtype=bf16)  # (2, 64)
    ones42 = np.array([[1, 0], [1, 0], [0, 1], [0, 1]], dtype=bf16)
    ones1x64 = np.ones((1, 64), dtype=f32)

    bhead = (np.asarray(bv, np.float64) + np.asarray(bav, np.float64)[None, :]).reshape(H * DEPTH)
    cvec = bhead @ np.asarray(Wd, np.float64) + np.asarray(bd, np.float64)
    cv128 = np.broadcast_to(cvec.astype(f32), (128, D)).copy()

    in_maps = []
    for ci in range(NCORES):
        b, ih = ci // 2, ci % 2
        i0 = ih * IH
        ii = pos[i0:i0 + IH]
        sin_i, cos_i = np.sin(CFREQ * ii), np.cos(CFREQ * ii)
        trigq4 = np.stack([cos_i, sin_i, -sin_i, cos_i]).astype(f32)   # for A,B
        trigP4 = np.stack([cos_i, -sin_i, sin_i, cos_i]).astype(f32)   # for P',Q'
        in_maps.append({
            "q": np.ascontiguousarray(q[b, i0:i0 + IH]).astype(bf16),
            "k": np.ascontiguousarray(k[b]).astype(bf16),
            "x": np.ascontiguousarray(x[b]).astype(bf16),
            "Wq": W2q, "Wk": W2k, "Wv": W2v,
            "Wd": np.asarray(Wd).astype(bf16),
            "bq_cols": bq_cols,
            "G4": G4, "Wav_t": Wav_t,
            "trig_row2": trig_row2,
            "trigq4": trigq4, "trigP4": trigP4,
            "trig_cols_rep": trig_cols_rep,
            "ones42": ones42, "ones1x64": ones1x64,
            "cv128": cv128,
        })
    return in_maps


def kernel(**inputs):
    from concourse import bass_utils

    x = np.asarray(inputs["x"]); k = np.asarray(inputs["k"]); q = np.asarray(inputs["q"])
    in_maps = _host_inputs(
        x, k, q,
        np.asarray(inputs["Wq"]), np.asarray(inputs["bq"]),
        np.asarray(inputs["Wk"]), np.asarray(inputs["bk"]),
        np.asarray(inputs["Wv"]), np.asarray(inputs["bv"]),
        np.asarray(inputs["Wak"]), np.asarray(inputs["bak"]),
        np.asarray(inputs["Wav"]), np.asarray(inputs["bav"]),
        np.asarray(inputs["Wd"]), np.asarray(inputs["bd"]),
    )
    if "prog" not in _PROGRAM_CACHE:
        _PROGRAM_CACHE["prog"] = _build_program()
    nc = _PROGRAM_CACHE["prog"]
    res = bass_utils.run_bass_kernel_spmd(nc, in_maps, core_ids=list(range(NCORES)))
    out = np.empty((B, L, D), dtype=np.float32)
    for ci in range(NCORES):
        b, ih = ci // 2, ci % 2
        out[b, ih * IH:(ih + 1) * IH, :] = res.results[ci]["out"]
    return out
